# revision 13
# baseline (speedup 1.0000x reference)
"""DND-LSTM cell (retrieval kNN + LSTM gates) on 8 Trainium2 NeuronCores.

Strategy (sharding_hint): shard keys/vals along dict_len (L=100000) across the
8 cores, 12500 each (padded to 12544 with dummy unit keys, excluded from the
softmax sums via ragged matmul slices). Each core streams its keysT/vals shard
from HBM once (memory-bound regime) and computes, flash-softmax style:

  num_partial[b, h]  = sum_l exp(cos(q_b, k_l) - 1) * vals[l, h]
  den_partial[b]     = sum_l exp(cos(q_b, k_l) - 1)

(cosine <= 1 exactly, so "-1" replaces the running row-max of a standard
streaming softmax; num/den ratios are unchanged.) The small LSTM-gate GEMMs are
sharded over the hidden dim (each core computes the 5 gate slices for its 32
hidden columns). The host gathers: sums the 8 num/den partials (the all-reduce)
and applies the final elementwise combine.

Device dataflow per 2048-key block (per core):
  DMA keysT [2x128d, 2048l] fp32 + vals [128, 16, 257] fp32 (257th col = 1.0
  feeds the denominator through the same matmul as the weighted val-sum)
  DVE:  sq = keysT * keysT
  PE :  normsq[1, l] = ones.T @ sq            (fp32r matmul, N=512 chunks)
  DMA:  scatter normsq [1,512] -> [128, 4]    (per-partition layout for ACT)
  ACT:  rsq = exp(-0.5 * ln(normsq))          (rsqrt; Ln+Exp share one ACT
                                               table -> no table thrash)
  PE :  simsT[l, b] = keysT_tile.T @ qnT      (per 128-l tile, fp32r)
  ACT:  expT = exp(simsT * rsq[l] - 1)        (fused per-partition scale+bias)
  PE :  av[b, 0:257] += expT_bhalf.T @ vals_tile   (persistent PSUM accum)

All matmuls use float32r (full-rate at N>=256). The only host arithmetic is
the 8-way partial sum + final elementwise LSTM combine (~0.002% of FLOPs).
"""

import numpy as np

import concourse.bacc as bacc
import concourse.masks as masks
import concourse.mybir as mybir
import concourse.tile as tile
from concourse import bass_utils

F32 = mybir.dt.float32
F32R = mybir.dt.float32r
AF = mybir.ActivationFunctionType

B = 256
D = 256
H = 256
NCORES = 8
HS = H // NCORES          # 32 hidden cols per core
GS = 5 * HS               # 160 gate cols per core
L = 100000
L_LOC = L // NCORES       # 12500 real keys per core
BLK = 2048                # keys per stream block
LPAD = ((L_LOC + 127) // 128) * 128  # 12544
NT_MAX = BLK // 128       # 16 l-tiles per full block


def _build(l_real=L_LOC, lpad=LPAD, blk=BLK):
    """Emit the per-core Bass program (identical on all 8 cores; all per-core
    variation is in the input data)."""
    nt_max = blk // 128
    nblk_full = lpad // blk
    tail = lpad - nblk_full * blk
    blocks = [blk] * nblk_full + ([tail] if tail else [])

    nc = bacc.Bacc("TRN2", target_bir_lowering=False, debug=False,
                   num_devices=NCORES)

    keysT = nc.dram_tensor("keysT", [D, lpad], F32R, kind="ExternalInput")
    vals = nc.dram_tensor("vals", [lpad, H], F32R, kind="ExternalInput")
    x_t = nc.dram_tensor("x_t", [B, D], F32, kind="ExternalInput")
    xT_aug = nc.dram_tensor("xT_aug", [D + 2, B], F32R, kind="ExternalInput")
    hT = nc.dram_tensor("hT", [H, B], F32R, kind="ExternalInput")
    WiT = nc.dram_tensor("WiT", [D + 2, GS], F32R, kind="ExternalInput")
    WhT = nc.dram_tensor("WhT", [H, GS], F32R, kind="ExternalInput")
    c_sl = nc.dram_tensor("c_sl", [B, HS], F32, kind="ExternalInput")
    onesc = nc.dram_tensor("onesc", [128, 32], F32R, kind="ExternalInput")

    nd = nc.dram_tensor("nd", [B, H + 2], F32, kind="ExternalOutput")
    org = nc.dram_tensor("org", [B, 3 * HS], F32, kind="ExternalOutput")

    with tile.TileContext(nc) as tc:
        with (
            tc.tile_pool(name="const", bufs=1) as const,
            tc.tile_pool(name="sbA", bufs=2) as sbA,
            tc.tile_pool(name="psA", bufs=1, space="PSUM") as psA,
            tc.tile_pool(name="kpool", bufs=2) as kpool,
            tc.tile_pool(name="sqpool", bufs=2) as sqpool,
            tc.tile_pool(name="nqps", bufs=2, space="PSUM") as nqps,
            tc.tile_pool(name="rqpool", bufs=2) as rqpool,
            tc.tile_pool(name="smps", bufs=3, space="PSUM") as smps,
            tc.tile_pool(name="expool", bufs=4) as expool,
            tc.tile_pool(name="avps", bufs=1, space="PSUM") as avps,
            tc.tile_pool(name="vring", bufs=1) as vring,
        ):
            # --- constants ---
            ident = const.tile([128, 128], F32)
            masks.make_identity(nc, ident[:])
            ones32 = const.tile([128, 32], F32R)
            nc.sync.dma_start(ones32[:], onesc.ap()[:])
            cm1 = const.tile([128, 1], F32)
            nc.vector.memset(cm1[:], -1.0)
            cm2 = const.tile([128, 1], F32)
            nc.vector.memset(cm2[:], -2.0)
            cm05 = const.tile([128, 1], F32)
            nc.vector.memset(cm05[:], -0.5)

            # persistent vals ring: [128, nt, 257]; col 256 = 1.0 (denominator)
            vbufs = []
            for i in range(3):
                vb = vring.tile([128, nt_max, H + 2], F32R, tag=f"vb{i}",
                                name=f"vb{i}")
                nc.sync.dma_start(
                    vb[:, 0:nt_max, H:H + 2],
                    onesc.ap()[:, 0:2 * nt_max].rearrange(
                        "p (t o) -> p t o", o=2))
                vbufs.append(vb)

            # --- phase A: qn = x / ||x||, then qnT via PE transpose ---
            qnT = [const.tile([128, B], F32R, tag=f"qnT{dc}", name=f"qnT{dc}")
                   for dc in range(2)]
            for bh in range(2):
                xt = sbA.tile([128, D], F32, tag="xt")
                nc.sync.dma_start(xt[:], x_t.ap()[bh * 128:(bh + 1) * 128, :])
                scr = sbA.tile([128, D], F32, tag="scr")
                nsq = sbA.tile([128, 1], F32, tag="nsq")
                nc.scalar.activation(scr[:], xt[:], AF.Square,
                                     accum_out=nsq[:])
                lnx = sbA.tile([128, 1], F32, tag="lnx")
                nc.scalar.activation(lnx[:], nsq[:], AF.Ln)
                rsx = sbA.tile([128, 1], F32, tag="rsx")
                nc.scalar.activation(rsx[:], lnx[:], AF.Exp, scale=cm05[:])
                qn = sbA.tile([128, D], F32, tag="qn")
                nc.vector.tensor_scalar_mul(qn[:], xt[:], rsx[:])
                for dc in range(2):
                    tp = psA.tile([128, 128], F32, tag="ps_scratch", name="tp")
                    nc.tensor.transpose(
                        tp[:], qn[:, dc * 128:(dc + 1) * 128], ident[:])
                    nc.vector.tensor_copy(
                        qnT[dc][:, bh * 128:(bh + 1) * 128], tp[:])

            # --- phase B: LSTM gate slices (this core's 32 hidden cols) ---
            xa = [sbA.tile([128, B], F32R, tag=f"xa{i}", name=f"xa{i}")
                  for i in range(2)]
            xa2 = sbA.tile([2, B], F32R, tag="xa2")
            ha = [sbA.tile([128, B], F32R, tag=f"ha{i}", name=f"ha{i}")
                  for i in range(2)]
            wi = [sbA.tile([128, GS], F32R, tag=f"wi{i}", name=f"wi{i}")
                  for i in range(2)]
            wi2 = sbA.tile([2, GS], F32R, tag="wi2")
            wh = [sbA.tile([128, GS], F32R, tag=f"wh{i}", name=f"wh{i}")
                  for i in range(2)]
            ctile = [sbA.tile([128, HS], F32, tag=f"ct{i}", name=f"ct{i}")
                     for i in range(2)]
            for i in range(2):
                nc.sync.dma_start(xa[i][:], xT_aug.ap()[i * 128:(i + 1) * 128, :])
                nc.sync.dma_start(ha[i][:], hT.ap()[i * 128:(i + 1) * 128, :])
                nc.sync.dma_start(wi[i][:], WiT.ap()[i * 128:(i + 1) * 128, :])
                nc.sync.dma_start(wh[i][:], WhT.ap()[i * 128:(i + 1) * 128, :])
                nc.sync.dma_start(
                    ctile[i][:], c_sl.ap()[i * 128:(i + 1) * 128, :])
            nc.sync.dma_start(xa2[:], xT_aug.ap()[256:258, :])
            nc.sync.dma_start(wi2[:], WiT.ap()[256:258, :])

            for bh in range(2):
                bsl = slice(bh * 128, (bh + 1) * 128)
                pre = psA.tile([128, GS], F32, tag="ps_scratch", name="pre")
                nc.tensor.matmul(pre[:], xa[0][:, bsl], wi[0][:],
                                 start=True, stop=False)
                nc.tensor.matmul(pre[:], xa[1][:, bsl], wi[1][:],
                                 start=False, stop=False)
                nc.tensor.matmul(pre[:], xa2[:, bsl], wi2[:],
                                 start=False, stop=False)
                nc.tensor.matmul(pre[:], ha[0][:, bsl], wh[0][:],
                                 start=False, stop=False)
                nc.tensor.matmul(pre[:], ha[1][:, bsl], wh[1][:],
                                 start=False, stop=True)
                gates = sbA.tile([128, GS], F32, tag="gates")
                # sigmoid(x) = exp(-ln(1 + exp(-x))): stays on the Ln/Exp ACT
                # table and avoids custom DVE ucode (reciprocal) entirely
                e1 = sbA.tile([128, 128], F32, tag="e1")
                nc.scalar.activation(e1[:], pre[:, 0:128], AF.Exp, scale=cm1[:])
                nc.vector.tensor_scalar_add(e1[:], e1[:], 1.0)
                l1 = sbA.tile([128, 128], F32, tag="l1")
                nc.scalar.activation(l1[:], e1[:], AF.Ln)
                nc.scalar.activation(gates[:, 0:128], l1[:], AF.Exp,
                                     scale=cm1[:])
                # tanh(x) = 2 * sigmoid(2x) - 1
                e2 = sbA.tile([128, HS], F32, tag="e2")
                nc.scalar.activation(e2[:], pre[:, 128:160], AF.Exp,
                                     scale=cm2[:])
                nc.vector.tensor_scalar_add(e2[:], e2[:], 1.0)
                l2 = sbA.tile([128, HS], F32, tag="l2")
                nc.scalar.activation(l2[:], e2[:], AF.Ln)
                e3 = sbA.tile([128, HS], F32, tag="e3")
                nc.scalar.activation(e3[:], l2[:], AF.Exp, scale=cm1[:])
                nc.vector.tensor_scalar(
                    gates[:, 128:160], e3[:], 2.0, -1.0,
                    op0=mybir.AluOpType.mult, op1=mybir.AluOpType.add)
                # c_part = f*c + i*c~
                fc = sbA.tile([128, HS], F32, tag="fc")
                nc.vector.tensor_mul(fc[:], gates[:, 0:HS], ctile[bh][:])
                ic = sbA.tile([128, HS], F32, tag="ic")
                nc.vector.tensor_mul(ic[:], gates[:, HS:2 * HS],
                                     gates[:, 128:160])
                cp = sbA.tile([128, HS], F32, tag="cp")
                nc.vector.tensor_add(cp[:], fc[:], ic[:])
                nc.sync.dma_start(org.ap()[bsl, 0:HS],
                                  gates[:, 2 * HS:3 * HS])      # o
                nc.sync.dma_start(org.ap()[bsl, HS:2 * HS],
                                  gates[:, 3 * HS:4 * HS])      # r
                nc.sync.dma_start(org.ap()[bsl, 2 * HS:3 * HS], cp[:])

            # --- phase C: stream the kNN retrieval ---
            av = [avps.tile([128, H + 2], F32, tag=f"av{bh}", name=f"av{bh}")
                  for bh in range(2)]
            total_tiles = lpad // 128
            tile_idx = 0
            for bi, bs in enumerate(blocks):
                off = bi * blk
                nt = bs // 128
                kts = []
                for dc in range(2):
                    kt = kpool.tile([128, bs], F32R, tag=f"kt{dc}")
                    nc.sync.dma_start(
                        kt[:], keysT.ap()[dc * 128:(dc + 1) * 128,
                                          off:off + bs])
                    kts.append(kt)
                vb = vbufs[bi % 3]
                nc.sync.dma_start(
                    vb[:, 0:nt, 0:H],
                    vals.ap()[off:off + bs, :].rearrange(
                        "(t p) h -> p t h", p=128))
                sqs = []
                for dc in range(2):
                    sq = sqpool.tile([128, bs], F32R, tag=f"sq{dc}")
                    nc.vector.tensor_mul(sq[:], kts[dc][:], kts[dc][:])
                    sqs.append(sq)
                # normsq[l] -> rsq[p, t] (= 1/||k_l||, l = 128*t + p) without a
                # 4-byte-descriptor scatter: chunk sums land on psum partitions
                # 32j (tile_position), bounce through SBUF, reshape-DMA to
                # [nt, 128], then one tiny PE transpose to [128, nt].
                chunks = [(j0, min(512, bs - j0)) for j0 in range(0, bs, 512)]
                t4 = rqpool.tile([16, 128], F32, tag="t4")
                for j, (j0, cs) in enumerate(chunks):
                    nq = nqps.tile([1, 512], F32, tag="nq")
                    nc.tensor.matmul(nq[:, 0:cs], ones32[:, 0:1],
                                     sqs[0][:, j0:j0 + cs],
                                     start=True, stop=False)
                    nc.tensor.matmul(nq[:, 0:cs], ones32[:, 0:1],
                                     sqs[1][:, j0:j0 + cs],
                                     start=False, stop=True)
                    nqs = rqpool.tile([1, 512], F32, tag="nqs")
                    if j % 2 == 0:
                        nc.vector.tensor_copy(nqs[:, 0:cs], nq[:, 0:cs])
                    else:
                        nc.scalar.copy(nqs[:, 0:cs], nq[:, 0:cs])
                    r = cs // 128
                    nc.sync.dma_start(
                        t4[4 * j:4 * j + r, :],
                        nqs[0:1, 0:cs].rearrange("o (r p) -> o r p", p=128))
                tpn = psA.tile([128, 16], F32, tag="ps_scratch", name="tpn")
                nc.tensor.transpose(tpn[:, 0:nt], t4[0:nt, :],
                                    ident[0:nt, 0:nt])
                rsq = rqpool.tile([128, nt_max], F32, tag="rsq")
                rln = rqpool.tile([128, nt_max], F32, tag="rln")
                nc.scalar.activation(rln[:, 0:nt], tpn[:, 0:nt], AF.Ln)
                nc.scalar.activation(rsq[:, 0:nt], rln[:, 0:nt], AF.Exp,
                                     scale=cm05[:])
                for t in range(nt):
                    l0 = off + t * 128
                    real = min(128, max(0, l_real - l0))
                    sm = smps.tile([128, B], F32, tag="sm")
                    nc.tensor.matmul(sm[:], kts[0][:, t * 128:(t + 1) * 128],
                                     qnT[0][:], start=True, stop=False)
                    nc.tensor.matmul(sm[:], kts[1][:, t * 128:(t + 1) * 128],
                                     qnT[1][:], start=False, stop=True)
                    ex = expool.tile([128, B], F32R, tag="ex")
                    nc.scalar.activation(ex[:], sm[:], AF.Exp,
                                         bias=cm1[:], scale=rsq[:, t:t + 1])
                    if real <= 0:
                        tile_idx += 1
                        continue
                    first = tile_idx == 0
                    last = tile_idx == total_tiles - 1
                    for bh in range(2):
                        nc.tensor.matmul(
                            av[bh][:],
                            ex[0:real, bh * 128:(bh + 1) * 128],
                            vb[0:real, t, :],
                            start=first, stop=last)
                    tile_idx += 1

            for bh in range(2):
                avs = sbA.tile([128, H + 2], F32, tag="avs")
                nc.vector.tensor_copy(avs[:], av[bh][:])
                nc.sync.dma_start(nd.ap()[bh * 128:(bh + 1) * 128, :],
                                  avs[:])

    nc.compile()
    return nc


_NC_CACHE = {}


def _get_nc():
    if "nc" not in _NC_CACHE:
        _NC_CACHE["nc"] = _build()
    return _NC_CACHE["nc"]


def _shard_inputs(x_t, h, c, W_i2h, b_i2h, W_h2h, b_h2h, keys, vals):
    f = np.float32
    x_t = np.ascontiguousarray(np.asarray(x_t, f))
    h = np.asarray(h, f)
    c = np.asarray(c, f)
    W_i2h = np.asarray(W_i2h, f)
    b_i2h = np.asarray(b_i2h, f)
    W_h2h = np.asarray(W_h2h, f)
    b_h2h = np.asarray(b_h2h, f)
    keys = np.asarray(keys, f)
    vals = np.asarray(vals, f)

    xT_aug = np.ascontiguousarray(
        np.concatenate([x_t.T, np.ones((2, B), f)], axis=0))
    hT = np.ascontiguousarray(h.T)
    WiT_full = W_i2h.T  # [D, G]
    WhT_full = W_h2h.T  # [H, G]

    in_maps = []
    for k in range(NCORES):
        sl = slice(k * L_LOC, (k + 1) * L_LOC)
        keysT = np.zeros((D, LPAD), f)
        keysT[:, :L_LOC] = keys[sl].T
        keysT[0, L_LOC:] = 1.0  # dummy unit keys (excluded from the sums)
        vpad = np.zeros((LPAD, H), f)
        vpad[:L_LOC] = vals[sl]
        gcols = np.concatenate(
            [np.arange(j * H + k * HS, j * H + (k + 1) * HS)
             for j in range(5)])
        WiT = np.concatenate(
            [WiT_full[:, gcols], b_i2h[gcols][None, :],
             b_h2h[gcols][None, :]], axis=0)
        in_maps.append({
            "onesc": np.ones((128, 32), f),
            "keysT": np.ascontiguousarray(keysT),
            "vals": np.ascontiguousarray(vpad),
            "x_t": x_t,
            "xT_aug": xT_aug,
            "hT": hT,
            "WiT": np.ascontiguousarray(WiT),
            "WhT": np.ascontiguousarray(WhT_full[:, gcols]),
            "c_sl": np.ascontiguousarray(c[:, k * HS:(k + 1) * HS]),
        })
    return in_maps


def kernel(x_t, h, c, W_i2h, b_i2h, W_h2h, b_h2h, keys, vals):
    nc = _get_nc()
    in_maps = _shard_inputs(x_t, h, c, W_i2h, b_i2h, W_h2h, b_h2h, keys, vals)
    res = bass_utils.run_bass_kernel_spmd(
        nc, in_maps, core_ids=list(range(NCORES)))

    num = np.zeros((B, H), np.float64)
    den = np.zeros((B,), np.float64)
    for k in range(NCORES):
        ndk = res.results[k]["nd"]
        num += ndk[:, :H]
        den += ndk[:, H]
    m = np.tanh(num / den[:, None]).astype(np.float32)

    h_t = np.empty((B, H), np.float32)
    c_t = np.empty((B, H), np.float32)
    for k in range(NCORES):
        orgk = res.results[k]["org"]
        o = orgk[:, 0:HS]
        r = orgk[:, HS:2 * HS]
        cp = orgk[:, 2 * HS:3 * HS]
        hs = slice(k * HS, (k + 1) * HS)
        ct = cp + r * m[:, hs]
        c_t[:, hs] = ct
        h_t[:, hs] = o * np.tanh(ct)
    return (h_t, c_t)
